# revision 7
# baseline (speedup 1.0000x reference)
"""FDLoss kernel for Trainium2 (Bass/Tile), data-parallel over 8 NeuronCores.

Math (a = target.flatten(), b = source.flatten()):
    fdback = where(a<0 & b<0, b-a, a-b)
    loss   = mean((fdback - a)^2)
Per element (case analysis):
    value = (b + relu(-2a) * (b<0))^2
The whole per-element pipeline + free-dim sum runs as ONE custom DVE op per
tile:  body = sq(Src1 + relu(Src0*C0)*(Src1 < Zero)), accum=add
(in0 = a half-tile, in1 = b half-tile, s0 = -2.0), accum_out -> acc[:, col].

Host-side, each core's shard is repacked so every tile is one contiguous
[rows, 2, n] block ([a-row | b-row] per partition) — one linear DMA per
partition-range, spread across the two HWDGE rings (SP and ACT) by a greedy
byte-balance.

SDMA engine 15 (serving partitions 92-95 and 124-127 via the port swizzle)
intermittently runs at ~21.5 GB/s instead of ~27 GB/s (seen in 2/3 profiled
runs, same magnitude each time; a known trn2 trait — engines 7/15 are often
slow).  Since every full-width DMA splits evenly over the 16 engines, that
one engine paces the whole kernel (+28us).  Counter: two tile types —
  F-tiles [128p x 3584c]: all partitions (18 of them incl. taper), and
  Z-tiles [120p x 2048c]: partitions [0:92]+[96:124] only (5 of them),
so engine 15 carries 40,576 cols/partition vs 50,816 for the other engines
(ratio 0.80 ~= 21.5/27).  On straggler runs both finish at ~121us; on clean
runs engine 15 just idles ~24us at the end with zero cost to the others.
Z-tiles keep full-128-partition DVE ops (partition-offset custom-DVE accum
writes corrupt other SBUF cells — observed on HW): engine 15's rows are
memset to 0 once per Z-buf and never DMA'd, contributing 0 to the sums.

Chunk schedule ends in a geometric taper of F-tiles (1280,1280 ... 128,128)
so the DVE's in-order queue drains almost in lockstep with the final DMA
bytes.  The partials store is split: columns 0..N-2 flush while the tail
streams; only the last 1-column store sits on the critical path after the
final (0.15us) DVE op.

Each core writes a [128, N_COLS] partial-sum tile (memset to zero first so
the Z-columns' unused partition rows read 0); the host sums the 8 small
tiles in f64 and divides by N (the output is a scalar, so a host-side gather
replaces the all-reduce in the sharding hint).
"""

from operator import add as _operator_add

import numpy as np

import concourse.bacc as bacc
import concourse.mybir as mybir
import concourse.dve_ops as dve_ops
from concourse.dve_ops import DveOp
from concourse.dve_spec import Spec, Src0, Src1, C0, Zero, relu, sq, lower, _has_src1
from concourse.dve_uop import DveOpSpec
from concourse.tile import TileContext
from concourse.bass_utils import run_bass_kernel_spmd

N_CORES = 8
FULL_SHAPE = (64, 256, 56, 56)
TOTAL = 64 * 256 * 56 * 56          # 51,380,224
PER_CORE = TOTAL // N_CORES         # 6,422,528 = 128 * 50,176
P = 128
FD_TOTAL = PER_CORE // P            # 50,176
FD = 3584                           # full F-tile column count
NZ = 2048                           # Z-tile column count

# partitions served by SDMA engine 15 (the intermittently-slow one)
DEAD = ((92, 96), (124, 128))
ZRANGES = ((0, 92), (96, 124))      # partition ranges carrying Z-tile data
N_FAST = 120                        # partitions in ZRANGES
N_SLOW = 8

N_F = 40576                         # cols/partition delivered via F-tiles
N_ZC = 5 * NZ                       # extra cols for fast partitions (Z-tiles)
assert 128 * N_F + N_FAST * N_ZC == TOTAL // N_CORES

_TAPER = [1280, 1280, 640, 640, 320, 320, 128, 128]
assert 10 * FD + sum(_TAPER) == N_F

# chunk plan: ('F', n) covers all 128 partitions; ('Z', n) covers ZRANGES.
# Z-tiles sit in the body (every 3rd slot); the taper is all-F so the end of
# the stream keeps every fast engine busy.
CHUNK_PLAN = []
for _i in range(5):
    CHUNK_PLAN += [("F", FD), ("F", FD), ("Z", NZ)]
CHUNK_PLAN += [("F", n) for n in _TAPER]
N_COLS = len(CHUNK_PLAN)            # 23
assert sum(n for t, n in CHUNK_PLAN if t == "F") == N_F
assert sum(n for t, n in CHUNK_PLAN if t == "Z") == N_ZC

# greedy byte-balance across the two HWDGE rings (0 = ACT/scalar, 1 = SP/sync)
RING = []
_rb = [0, 0]
for _i, (_t, _n) in enumerate(CHUNK_PLAN):
    _bytes = (128 if _t == "F" else N_FAST) * 2 * _n * 4
    r = 0 if _rb[0] <= _rb[1] else 1
    RING.append(r)
    _rb[r] += _bytes

_F32 = mybir.dt.float32

_OP_NAME = "FDLOSS_SQ_REDUCE"


def _fdloss_ref(in0, in1, c0, c1, c2):
    """CoreSim reference: (out, accum_out) for the accum-bearing spec."""
    b = np.square(
        in1 + np.maximum(in0 * c0, 0.0) * (in1 < 0.0)
    ).astype(np.float32)
    return b, b.reshape(b.shape[0], -1).sum(axis=-1, keepdims=True)


def _register_op() -> DveOp:
    """Register the fused op in dve_ops' registries (repo is read-only, so we
    extend OPS at runtime — same effect as adding the constant in the file)."""
    for op in dve_ops.OPS:
        if op.name == _OP_NAME:
            return op
    spec = Spec(
        body=sq(Src1 + relu(Src0 * C0) * (Src1 < Zero)),
        accum=_operator_add,
        accum_init=Zero,
        reference=_fdloss_ref,
    )
    row = dve_ops._CUSTOM_DVE_ROW_BASE + len(dve_ops.OPS)
    shas = {}
    for ver in ("v3", "v4"):
        compiled = DveOpSpec(
            name=_OP_NAME,
            opcode=row,
            uops=lower(spec, ver=ver),
            rd1_en=_has_src1(spec),
        )
        shas[ver] = compiled.sha(ver)
    op = DveOp(_OP_NAME, spec, subdim=False, uops_sha=shas)
    dve_ops.OPS.append(op)
    dve_ops._SUB_OPCODE_FOR_NAME[_OP_NAME] = row
    dve_ops.CUSTOM_DVE_SPECS[_OP_NAME] = spec
    return op


_cached_nc = None


def _build_bass():
    """Build the single-core SPMD Bass program (same NEFF on all 8 cores)."""
    fd_op = _register_op()
    nc = bacc.Bacc(trn_type="TRN2")

    ab_d = nc.dram_tensor("ab_in", (2 * PER_CORE,), _F32, kind="ExternalInput")
    out_d = nc.dram_tensor("partials", (P, N_COLS), _F32, kind="ExternalOutput")

    with TileContext(nc) as tc:
        with (
            tc.tile_pool(name="ab", bufs=5) as ab_pool,
            tc.tile_pool(name="z", bufs=2) as z_pool,
            tc.tile_pool(name="w", bufs=1) as w_pool,
            tc.tile_pool(name="acc", bufs=1) as acc_pool,
        ):
            acc = acc_pool.tile([P, N_COLS], _F32)
            wt = w_pool.tile([P, FD], _F32)  # write-only scratch for `out`
            # zero acc so Z-columns' never-written rows (92:96, 124:128)
            # contribute 0 to the host-side sum
            nc.vector.memset(acc[:], 0.0)
            elem_off = 0
            zi = 0
            for i, (typ, n) in enumerate(CHUNK_PLAN):
                dma_eng = nc.scalar if RING[i] == 0 else nc.sync
                if typ == "F":
                    abt = ab_pool.tile([P, 2 * FD], _F32, tag="ab")
                    src = ab_d[elem_off : elem_off + P * 2 * n].rearrange(
                        "(p m) -> p m", p=P
                    )
                    elem_off += P * 2 * n
                    dma_eng.dma_start(out=abt[:, : 2 * n], in_=src)
                    nc.vector._custom_dve(
                        fd_op,
                        out=wt[:, :n],
                        in0=abt[:, :n],
                        in1=abt[:, n : 2 * n],
                        s0=-2.0,
                        accum_out=acc[:, i : i + 1],
                    )
                else:
                    zt = z_pool.tile([P, 2 * NZ], _F32, tag="z")
                    if zi < 2:
                        # first use of each of the 2 Z-pool bufs: zero the
                        # whole tile once (compute-op partition bases must be
                        # multiples of 32, so target all 128 rows; the DMAs
                        # below overwrite the live rows).  Later Z-chunks
                        # never write the dead rows, so the zeros persist
                        # across buf reuse and the full-128-partition DVE op
                        # sees 0 there -> adds 0.
                        nc.vector.memset(zt[:], 0.0)
                    zi += 1
                    for r0, r1 in ZRANGES:
                        rows = r1 - r0
                        src = ab_d[
                            elem_off : elem_off + rows * 2 * n
                        ].rearrange("(p m) -> p m", p=rows)
                        elem_off += rows * 2 * n
                        dma_eng.dma_start(out=zt[r0:r1, : 2 * n], in_=src)
                    nc.vector._custom_dve(
                        fd_op,
                        out=wt[:, :n],
                        in0=zt[:, :n],
                        in1=zt[:, n : 2 * n],
                        s0=-2.0,
                        accum_out=acc[:, i : i + 1],
                    )
                if i == N_COLS - 2:
                    # flush all but the last partials column while the tail
                    # chunk is still streaming (reads cols 0..N-2)
                    nc.sync.dma_start(
                        out=out_d[:, : N_COLS - 1], in_=acc[:, : N_COLS - 1]
                    )
            # only this 1-column store trails the final DVE op
            nc.scalar.dma_start(
                out=out_d[:, N_COLS - 1 :], in_=acc[:, N_COLS - 1 :]
            )
            assert elem_off == 2 * PER_CORE

    nc.compile()
    return nc


def _get_nc():
    global _cached_nc
    if _cached_nc is None:
        _cached_nc = _build_bass()
    return _cached_nc


def _partition_slices():
    """Flat per-core [PER_CORE] array -> per-partition contiguous streams.
    Fast partitions (not DEAD) get N_F + N_ZC elements; DEAD ones get N_F."""
    dead = np.zeros(P, dtype=bool)
    for r0, r1 in DEAD:
        dead[r0:r1] = True
    lengths = np.where(dead, N_F, N_F + N_ZC)
    starts = np.zeros(P, dtype=np.int64)
    starts[1:] = np.cumsum(lengths)[:-1]
    return starts, lengths


def _pack_inputs(source, target):
    """Repack full inputs into per-core flat [2*PER_CORE] arrays matching the
    chunk plan: each DMA's source is a contiguous [rows, 2, n] block."""
    a = np.asarray(target, dtype=np.float32).reshape(N_CORES, PER_CORE)
    b = np.asarray(source, dtype=np.float32).reshape(N_CORES, PER_CORE)
    starts, _ = _partition_slices()
    packed = np.empty((N_CORES, 2 * PER_CORE), dtype=np.float32)
    col = np.arange(0)  # placeholder
    off_f = 0   # cols consumed by F-chunks (all partitions)
    off_z = 0   # cols consumed by Z-chunks (fast partitions only)
    elem_off = 0
    for typ, n in CHUNK_PLAN:
        if typ == "F":
            ranges = ((0, P),)
            base = off_f
        else:
            ranges = ZRANGES
            base = N_F + off_z
        for r0, r1 in ranges:
            rows = r1 - r0
            idx = starts[r0:r1, None] + base + np.arange(n)[None, :]
            blk = np.stack([a[:, idx], b[:, idx]], axis=2)  # [C, rows, 2, n]
            packed[:, elem_off : elem_off + rows * 2 * n] = blk.reshape(
                N_CORES, -1
            )
            elem_off += rows * 2 * n
        if typ == "F":
            off_f += n
        else:
            off_z += n
    assert elem_off == 2 * PER_CORE
    return packed


def kernel_impl(source, target, trace=False, **run_kwargs):
    """Returns (loss_scalar_f32, BassKernelResults)."""
    packed = _pack_inputs(source, target)
    in_maps = [{"ab_in": packed[i]} for i in range(N_CORES)]

    nc = _get_nc()
    res = run_bass_kernel_spmd(
        nc, in_maps, core_ids=list(range(N_CORES)), trace=trace, **run_kwargs
    )
    total = np.float64(0.0)
    for r in res.results:
        total += r["partials"].astype(np.float64).sum()
    loss = np.float32(total / TOTAL)
    return np.array(loss, dtype=np.float32), res


def kernel(**inputs) -> np.ndarray:
    out, _ = kernel_impl(inputs["source"], inputs["target"])
    return out


# revision 9
# speedup vs baseline: 1.0788x; 1.0788x over previous
"""FDLoss kernel for Trainium2 (Bass/Tile), data-parallel over 8 NeuronCores.

Math (a = target.flatten(), b = source.flatten()):
    fdback = where(a<0 & b<0, b-a, a-b)
    loss   = mean((fdback - a)^2)
Per element (case analysis):
    value = (b + relu(-2a) * (b<0))^2
The whole per-element pipeline + free-dim sum runs as ONE custom DVE op per
tile:  body = sq(Src1 + relu(Src0*C0)*(Src1 < Zero)), accum=add
(in0 = a half-tile, in1 = b half-tile, s0 = -2.0), accum_out -> acc[:, col].

Host-side, each core's shard is repacked so every tile is one contiguous
[rows, 2, n] block ([a-row | b-row] per partition) — one linear DMA per
partition-range, spread across the two HWDGE rings (SP and ACT) by a greedy
byte-balance.

SDMA engine 15 (serving partitions 92-95 and 124-127 via the port swizzle)
intermittently runs at ~21.5 GB/s instead of ~27 GB/s (seen in 2/3 profiled
runs, same magnitude each time; a known trn2 trait — engines 7/15 are often
slow).  Since every full-width DMA splits evenly over the 16 engines, that
one engine paces the whole kernel (+28us).  Counter: two tile types —
  F-tiles [128p x 3584c]: all partitions (18 of them incl. taper), and
  Z-tiles [120p x 2048c]: partitions [0:92]+[96:124] only (5 of them),
so engine 15 carries 40,576 cols/partition vs 50,816 for the other engines
(ratio 0.80 ~= 21.5/27).  On straggler runs both finish at ~121us; on clean
runs engine 15 just idles ~24us at the end with zero cost to the others.
Z-tiles keep full-128-partition DVE ops (partition-offset custom-DVE accum
writes corrupt other SBUF cells — observed on HW): engine 15's rows are
memset to 0 once per Z-buf and never DMA'd, contributing 0 to the sums.

Chunk schedule ends in a geometric taper of F-tiles (1280,1280 ... 128,128)
so the DVE's in-order queue drains almost in lockstep with the final DMA
bytes.  The partials store is split: columns 0..N-2 flush while the tail
streams; only the last 1-column store sits on the critical path after the
final (0.15us) DVE op.

Each core writes a [128, N_COLS] partial-sum tile (memset to zero first so
the Z-columns' unused partition rows read 0); the host sums the 8 small
tiles in f64 and divides by N (the output is a scalar, so a host-side gather
replaces the all-reduce in the sharding hint).
"""

from operator import add as _operator_add

import numpy as np

import concourse.bacc as bacc
import concourse.mybir as mybir
import concourse.dve_ops as dve_ops
from concourse.dve_ops import DveOp
from concourse.dve_spec import Spec, Src0, Src1, C0, Zero, relu, sq, lower, _has_src1
from concourse.dve_uop import DveOpSpec
from concourse.tile import TileContext
from concourse.bass_utils import run_bass_kernel_spmd

N_CORES = 8
FULL_SHAPE = (64, 256, 56, 56)
TOTAL = 64 * 256 * 56 * 56          # 51,380,224
PER_CORE = TOTAL // N_CORES         # 6,422,528 = 128 * 50,176
P = 128
FD_TOTAL = PER_CORE // P            # 50,176
FD = 3584                           # full F-tile column count
NZ = 2048                           # Z-tile column count

# partitions served by SDMA engine 15 (the intermittently-slow one)
DEAD = ((92, 96), (124, 128))
ZRANGES = ((0, 92), (96, 124))      # partition ranges carrying Z-tile data
N_FAST = 120                        # partitions in ZRANGES
N_SLOW = 8

N_F = 40576                         # cols/partition delivered via F-tiles
N_ZC = 5 * NZ                       # extra cols for fast partitions (Z-tiles)
assert 128 * N_F + N_FAST * N_ZC == TOTAL // N_CORES

_TAPER = [1280, 1280, 640, 640, 320, 320, 128, 128]
assert 10 * FD + sum(_TAPER) == N_F

# chunk plan: ('F', n) covers all 128 partitions; ('Z', n) covers ZRANGES.
# Z-tiles sit in the body (every 3rd slot); the taper is all-F so the end of
# the stream keeps every fast engine busy.
CHUNK_PLAN = []
for _i in range(5):
    CHUNK_PLAN += [("F", FD), ("F", FD), ("Z", NZ)]
CHUNK_PLAN += [("F", n) for n in _TAPER]
N_COLS = len(CHUNK_PLAN)            # 23
assert sum(n for t, n in CHUNK_PLAN if t == "F") == N_F
assert sum(n for t, n in CHUNK_PLAN if t == "Z") == N_ZC

# greedy byte-balance across the two HWDGE rings (0 = ACT/scalar, 1 = SP/sync)
RING = []
_rb = [0, 0]
for _i, (_t, _n) in enumerate(CHUNK_PLAN):
    _bytes = (128 if _t == "F" else N_FAST) * 2 * _n * 4
    r = 0 if _rb[0] <= _rb[1] else 1
    RING.append(r)
    _rb[r] += _bytes

_F32 = mybir.dt.float32

_OP_NAME = "FDLOSS_SQ_REDUCE"


def _fdloss_ref(in0, in1, c0, c1, c2):
    """CoreSim reference: (out, accum_out) for the accum-bearing spec."""
    b = np.square(
        in1 + np.maximum(in0 * c0, 0.0) * (in1 < 0.0)
    ).astype(np.float32)
    return b, b.reshape(b.shape[0], -1).sum(axis=-1, keepdims=True)


def _register_op() -> DveOp:
    """Register the fused op in dve_ops' registries (repo is read-only, so we
    extend OPS at runtime — same effect as adding the constant in the file)."""
    for op in dve_ops.OPS:
        if op.name == _OP_NAME:
            return op
    spec = Spec(
        body=sq(Src1 + relu(Src0 * C0) * (Src1 < Zero)),
        accum=_operator_add,
        accum_init=Zero,
        reference=_fdloss_ref,
    )
    row = dve_ops._CUSTOM_DVE_ROW_BASE + len(dve_ops.OPS)
    shas = {}
    for ver in ("v3", "v4"):
        compiled = DveOpSpec(
            name=_OP_NAME,
            opcode=row,
            uops=lower(spec, ver=ver),
            rd1_en=_has_src1(spec),
        )
        shas[ver] = compiled.sha(ver)
    op = DveOp(_OP_NAME, spec, subdim=False, uops_sha=shas)
    dve_ops.OPS.append(op)
    dve_ops._SUB_OPCODE_FOR_NAME[_OP_NAME] = row
    dve_ops.CUSTOM_DVE_SPECS[_OP_NAME] = spec
    return op


_cached_nc = None


def _build_bass():
    """Build the single-core SPMD Bass program (same NEFF on all 8 cores)."""
    fd_op = _register_op()
    nc = bacc.Bacc(trn_type="TRN2")

    ab_d = nc.dram_tensor("ab_in", (2 * PER_CORE,), _F32, kind="ExternalInput")
    out_d = nc.dram_tensor("partials", (P, N_COLS), _F32, kind="ExternalOutput")

    with TileContext(nc) as tc:
        with (
            tc.tile_pool(name="ab", bufs=5) as ab_pool,
            tc.tile_pool(name="z", bufs=2) as z_pool,
            tc.tile_pool(name="w", bufs=1) as w_pool,
            tc.tile_pool(name="acc", bufs=1) as acc_pool,
        ):
            acc = acc_pool.tile([P, N_COLS], _F32)
            wt = w_pool.tile([P, FD], _F32)  # write-only scratch for `out`
            # zero acc so Z-columns' never-written rows (92:96, 124:128)
            # contribute 0 to the host-side sum
            nc.vector.memset(acc[:], 0.0)
            elem_off = 0
            zi = 0
            for i, (typ, n) in enumerate(CHUNK_PLAN):
                dma_eng = nc.scalar if RING[i] == 0 else nc.sync
                if typ == "F":
                    abt = ab_pool.tile([P, 2 * FD], _F32, tag="ab")
                    src = ab_d[elem_off : elem_off + P * 2 * n].rearrange(
                        "(p m) -> p m", p=P
                    )
                    elem_off += P * 2 * n
                    dma_eng.dma_start(out=abt[:, : 2 * n], in_=src)
                    nc.vector._custom_dve(
                        fd_op,
                        out=wt[:, :n],
                        in0=abt[:, :n],
                        in1=abt[:, n : 2 * n],
                        s0=-2.0,
                        accum_out=acc[:, i : i + 1],
                    )
                else:
                    zt = z_pool.tile([P, 2 * NZ], _F32, tag="z")
                    if zi < 2:
                        # first use of each of the 2 Z-pool bufs: zero the
                        # whole tile once (compute-op partition bases must be
                        # multiples of 32, so target all 128 rows; the DMAs
                        # below overwrite the live rows).  Later Z-chunks
                        # never write the dead rows, so the zeros persist
                        # across buf reuse and the full-128-partition DVE op
                        # sees 0 there -> adds 0.
                        nc.vector.memset(zt[:], 0.0)
                    zi += 1
                    for r0, r1 in ZRANGES:
                        rows = r1 - r0
                        src = ab_d[
                            elem_off : elem_off + rows * 2 * n
                        ].rearrange("(p m) -> p m", p=rows)
                        elem_off += rows * 2 * n
                        # SWDGE (gpsimd): partial-partition DMAs only touch
                        # 15 of the 16 SDMA engines, which breaks the HWDGE
                        # per-DMA completion accounting (each engine incs the
                        # sem once -> 15 incs where 16 are awaited, stalling
                        # consumers until a later DMA's incs arrive).  The
                        # software-DGE path controls its sem incs explicitly,
                        # so irregular shapes complete correctly — and it
                        # rides a third descriptor-generation queue.
                        nc.gpsimd.dma_start(out=zt[r0:r1, : 2 * n], in_=src)
                    nc.vector._custom_dve(
                        fd_op,
                        out=wt[:, :n],
                        in0=zt[:, :n],
                        in1=zt[:, n : 2 * n],
                        s0=-2.0,
                        accum_out=acc[:, i : i + 1],
                    )
            # flush all but the last partials column; emitted after every
            # dma_start so it cannot block a ring's FIFO mid-stream — it
            # overlaps the tail chunk's stream + DVE op
            nc.sync.dma_start(
                out=out_d[:, : N_COLS - 1], in_=acc[:, : N_COLS - 1]
            )
            # only this 1-column store trails the final DVE op
            nc.scalar.dma_start(
                out=out_d[:, N_COLS - 1 :], in_=acc[:, N_COLS - 1 :]
            )
            assert elem_off == 2 * PER_CORE

    nc.compile()
    return nc


def _get_nc():
    global _cached_nc
    if _cached_nc is None:
        _cached_nc = _build_bass()
    return _cached_nc


def _partition_slices():
    """Flat per-core [PER_CORE] array -> per-partition contiguous streams.
    Fast partitions (not DEAD) get N_F + N_ZC elements; DEAD ones get N_F."""
    dead = np.zeros(P, dtype=bool)
    for r0, r1 in DEAD:
        dead[r0:r1] = True
    lengths = np.where(dead, N_F, N_F + N_ZC)
    starts = np.zeros(P, dtype=np.int64)
    starts[1:] = np.cumsum(lengths)[:-1]
    return starts, lengths


def _pack_inputs(source, target):
    """Repack full inputs into per-core flat [2*PER_CORE] arrays matching the
    chunk plan: each DMA's source is a contiguous [rows, 2, n] block."""
    a = np.asarray(target, dtype=np.float32).reshape(N_CORES, PER_CORE)
    b = np.asarray(source, dtype=np.float32).reshape(N_CORES, PER_CORE)
    starts, _ = _partition_slices()
    packed = np.empty((N_CORES, 2 * PER_CORE), dtype=np.float32)
    col = np.arange(0)  # placeholder
    off_f = 0   # cols consumed by F-chunks (all partitions)
    off_z = 0   # cols consumed by Z-chunks (fast partitions only)
    elem_off = 0
    for typ, n in CHUNK_PLAN:
        if typ == "F":
            ranges = ((0, P),)
            base = off_f
        else:
            ranges = ZRANGES
            base = N_F + off_z
        for r0, r1 in ranges:
            rows = r1 - r0
            idx = starts[r0:r1, None] + base + np.arange(n)[None, :]
            blk = np.stack([a[:, idx], b[:, idx]], axis=2)  # [C, rows, 2, n]
            packed[:, elem_off : elem_off + rows * 2 * n] = blk.reshape(
                N_CORES, -1
            )
            elem_off += rows * 2 * n
        if typ == "F":
            off_f += n
        else:
            off_z += n
    assert elem_off == 2 * PER_CORE
    return packed


def kernel_impl(source, target, trace=False, **run_kwargs):
    """Returns (loss_scalar_f32, BassKernelResults)."""
    packed = _pack_inputs(source, target)
    in_maps = [{"ab_in": packed[i]} for i in range(N_CORES)]

    nc = _get_nc()
    res = run_bass_kernel_spmd(
        nc, in_maps, core_ids=list(range(N_CORES)), trace=trace, **run_kwargs
    )
    total = np.float64(0.0)
    for r in res.results:
        total += r["partials"].astype(np.float64).sum()
    loss = np.float32(total / TOTAL)
    return np.array(loss, dtype=np.float32), res


def kernel(**inputs) -> np.ndarray:
    out, _ = kernel_impl(inputs["source"], inputs["target"])
    return out


# revision 10
# speedup vs baseline: 1.7778x; 1.6479x over previous
"""FDLoss kernel for Trainium2 (Bass/Tile), data-parallel over 8 NeuronCores.

Math (a = target.flatten(), b = source.flatten()):
    fdback = where(a<0 & b<0, b-a, a-b)
    loss   = mean((fdback - a)^2)
Per element (case analysis):
    value = (b + relu(-2a) * (b<0))^2
The whole per-element pipeline + free-dim sum runs as ONE custom DVE op per
tile:  body = sq(Src1 + relu(Src0*C0)*(Src1 < Zero)), accum=add
(in0 = a half-tile, in1 = b half-tile, s0 = -2.0), accum_out -> acc[:, i].

Host-side, each core's shard is repacked so every tile is one contiguous
[P, 2*FD] block holding [a-row | b-row] per partition — one 3.67 MB linear
DMA per tile (one descriptor per partition; partial-partition DMA shapes
measured 1.6-1.8x slower end-to-end on this part, so every DMA covers all
128 partitions / all 16 SDMA engines), alternated across the two HWDGE
rings (SP and ACT).

Chunk schedule: 12 full 3584-col tiles, then a geometric taper in ring-pairs
(1792,1792, 896,896, 448,448, 280,280, 168,168) so the DVE's in-order queue
drains almost in lockstep with the final DMA bytes (a blunt 5-chunk tail
left a ~4us DVE overhang past the last DMA).  The partials store is split:
columns 0..N-2 flush concurrently with the tail chunks; only the last
1-column store sits on the critical path after the final (0.2us) DVE op.

Each core writes a [128, N_COLS] partial-sum tile; the host sums the 8 small
tiles in f64 and divides by N (the output is a scalar, so a host-side gather
replaces the all-reduce in the sharding hint).
"""

from operator import add as _operator_add

import numpy as np

import concourse.bacc as bacc
import concourse.mybir as mybir
import concourse.dve_ops as dve_ops
from concourse.dve_ops import DveOp
from concourse.dve_spec import Spec, Src0, Src1, C0, Zero, relu, sq, lower, _has_src1
from concourse.dve_uop import DveOpSpec
from concourse.tile import TileContext
from concourse.bass_utils import run_bass_kernel_spmd

N_CORES = 8
FULL_SHAPE = (64, 256, 56, 56)
TOTAL = 64 * 256 * 56 * 56          # 51,380,224
PER_CORE = TOTAL // N_CORES         # 6,422,528 = 128 * 50,176
P = 128
FD_TOTAL = PER_CORE // P            # 50,176
FD = 3584                           # full-tile column count

# 12 full tiles + paired taper (one of each size per HWDGE ring)
_TAPER = [1792, 1792, 896, 896, 448, 448, 280, 280, 168, 168]
CHUNK_SIZES = [FD] * 12 + _TAPER
assert sum(CHUNK_SIZES) == FD_TOTAL
CHUNKS = []
_off = 0
for _n in CHUNK_SIZES:
    CHUNKS.append((_off, _n))
    _off += _n
N_COLS = len(CHUNKS)                # 22

_F32 = mybir.dt.float32

_OP_NAME = "FDLOSS_SQ_REDUCE"


def _fdloss_ref(in0, in1, c0, c1, c2):
    """CoreSim reference: (out, accum_out) for the accum-bearing spec."""
    b = np.square(
        in1 + np.maximum(in0 * c0, 0.0) * (in1 < 0.0)
    ).astype(np.float32)
    return b, b.reshape(b.shape[0], -1).sum(axis=-1, keepdims=True)


def _register_op() -> DveOp:
    """Register the fused op in dve_ops' registries (repo is read-only, so we
    extend OPS at runtime — same effect as adding the constant in the file)."""
    for op in dve_ops.OPS:
        if op.name == _OP_NAME:
            return op
    spec = Spec(
        body=sq(Src1 + relu(Src0 * C0) * (Src1 < Zero)),
        accum=_operator_add,
        accum_init=Zero,
        reference=_fdloss_ref,
    )
    row = dve_ops._CUSTOM_DVE_ROW_BASE + len(dve_ops.OPS)
    shas = {}
    for ver in ("v3", "v4"):
        compiled = DveOpSpec(
            name=_OP_NAME,
            opcode=row,
            uops=lower(spec, ver=ver),
            rd1_en=_has_src1(spec),
        )
        shas[ver] = compiled.sha(ver)
    op = DveOp(_OP_NAME, spec, subdim=False, uops_sha=shas)
    dve_ops.OPS.append(op)
    dve_ops._SUB_OPCODE_FOR_NAME[_OP_NAME] = row
    dve_ops.CUSTOM_DVE_SPECS[_OP_NAME] = spec
    return op


_cached_nc = None


def _build_bass():
    """Build the single-core SPMD Bass program (same NEFF on all 8 cores)."""
    fd_op = _register_op()
    nc = bacc.Bacc(trn_type="TRN2")

    # packed layout: per core one flat [2*PER_CORE] tensor; chunk k occupies a
    # contiguous block of P*2*n_k elements laid out as [P, 2, n_k] (per
    # partition: a-row then b-row), so each tile is one linear DMA.
    ab_d = nc.dram_tensor("ab_in", (2 * PER_CORE,), _F32, kind="ExternalInput")
    out_d = nc.dram_tensor("partials", (P, N_COLS), _F32, kind="ExternalOutput")

    with TileContext(nc) as tc:
        with (
            tc.tile_pool(name="ab", bufs=6) as ab_pool,
            tc.tile_pool(name="w", bufs=1) as w_pool,
            tc.tile_pool(name="acc", bufs=1) as acc_pool,
        ):
            acc = acc_pool.tile([P, N_COLS], _F32)
            wt = w_pool.tile([P, FD], _F32)  # write-only scratch for `out`
            elem_off = 0
            for i, (off, n) in enumerate(CHUNKS):
                abt = ab_pool.tile([P, 2 * FD], _F32, tag="ab")
                # ACT ring first: SP's queue is busier with preamble/postamble
                # EvSems, so chunk 0 streams earlier from the ACT HWDGE ring.
                dma_eng = nc.scalar if i % 2 == 0 else nc.sync
                src = ab_d[elem_off : elem_off + P * 2 * n].rearrange(
                    "(p m) -> p m", p=P
                )
                elem_off += P * 2 * n
                dma_eng.dma_start(out=abt[:, : 2 * n], in_=src)
                nc.vector._custom_dve(
                    fd_op,
                    out=wt[:, :n],
                    in0=abt[:, :n],
                    in1=abt[:, n : 2 * n],
                    s0=-2.0,
                    accum_out=acc[:, i : i + 1],
                )
            # flush all but the last partials column; emitted after every
            # dma_start so it cannot block a ring's FIFO mid-stream — it
            # overlaps the tail chunk's stream + DVE op
            nc.sync.dma_start(
                out=out_d[:, : N_COLS - 1], in_=acc[:, : N_COLS - 1]
            )
            # only this 1-column store trails the final (0.2us) DVE op
            nc.scalar.dma_start(
                out=out_d[:, N_COLS - 1 :], in_=acc[:, N_COLS - 1 :]
            )

    nc.compile()
    return nc


def _get_nc():
    global _cached_nc
    if _cached_nc is None:
        _cached_nc = _build_bass()
    return _cached_nc


def _pack_inputs(source, target):
    """Repack full inputs into per-core flat [2*PER_CORE] arrays where chunk k
    is a contiguous [P, 2, n_k] block (a-row then b-row per partition)."""
    a = np.asarray(target, dtype=np.float32).reshape(N_CORES, P, FD_TOTAL)
    b = np.asarray(source, dtype=np.float32).reshape(N_CORES, P, FD_TOTAL)
    packed = np.empty((N_CORES, 2 * PER_CORE), dtype=np.float32)
    elem_off = 0
    for off, n in CHUNKS:
        blk = np.stack(
            [a[:, :, off : off + n], b[:, :, off : off + n]], axis=2
        )  # [C, P, 2, n]
        packed[:, elem_off : elem_off + P * 2 * n] = blk.reshape(N_CORES, -1)
        elem_off += P * 2 * n
    return packed


def kernel_impl(source, target, trace=False, **run_kwargs):
    """Returns (loss_scalar_f32, BassKernelResults)."""
    packed = _pack_inputs(source, target)
    in_maps = [{"ab_in": packed[i]} for i in range(N_CORES)]

    nc = _get_nc()
    res = run_bass_kernel_spmd(
        nc, in_maps, core_ids=list(range(N_CORES)), trace=trace, **run_kwargs
    )
    total = np.float64(0.0)
    for r in res.results:
        total += r["partials"].astype(np.float64).sum()
    loss = np.float32(total / TOTAL)
    return np.array(loss, dtype=np.float32), res


def kernel(**inputs) -> np.ndarray:
    out, _ = kernel_impl(inputs["source"], inputs["target"])
    return out
